# revision 1
# baseline (speedup 1.0000x reference)
"""FF-sharded expert-parallel MoE FFN kernel for Trainium2 (8 NeuronCores).

Strategy (hybrid expert-parallel x FF-tensor-parallel):
  - Host computes the gate in fp32 (softmax -> top-2 -> renormalize).
  - The 8 experts are split into 2 groups of 4 (paired so group token
    sums balance); each group owns 4 cores. Within a group, the FFN
    hidden dim (FF=4096) is sharded 4 ways: core (g, j) holds columns
    [j*1024, (j+1)*1024) of W1 and the matching rows of W2 for all 4 of
    group g's experts, and processes ALL of the group's routed tokens
    over that shard:
        Ypart = gelu(X @ W1[:, shard] + b1[shard]) @ W2[shard, :]
    The host sums the 4 partials per group, applies the top-2 combine
    weights, and adds the b2 term.
  - Why 4-way shards (not 8): per-core load is slot_tokens/4 which
    balances to within ~1% for any routing, while total HBM traffic
    stays at ~33MB/core (8-way sharding replicates x/y to every core
    and oversubscribes chip HBM bandwidth).

Per-core schedule (4 segments = group experts, descending size):
  Each segment's W1 shard and x tokens are packed side-by-side in ONE
  DRAM tensor ([w1 | x^T], rows = hidden dim), so segment inputs arrive
  with a handful of column-slice DMAs in exact consumption order; the
  kernel's first matmul needs only ~1MB of DMA. Segment s+1's inputs
  are issued right after GEMM1(s) - a full GEMM-phase of prefetch.
  All GEMMs in bf16 on the PE with fp32 PSUM accumulation; gelu (exact)
  is fused into the GEMM1 PSUM eviction with the b1 bias; GEMM2 partial
  outputs are written as bf16 (host sums partials in fp32).
"""

import sys

if "/opt/trn_rl_repo" not in sys.path:
    sys.path.insert(0, "/opt/trn_rl_repo")

import numpy as np
import ml_dtypes

H = 1024          # hidden size
E = 8             # experts
TOPK = 2
FF = 4 * H        # expert hidden dim
P = 128           # SBUF partitions
NG = 2            # expert groups
GE = E // NG      # experts per group (4) == cores per group == FF shards
FFS = FF // GE    # per-core FF shard (1024)
KH = H // P       # 8  contraction chunks for GEMM1
KFS = FFS // P    # 8  contraction chunks for GEMM2 (shard)

_prog_cache: dict[tuple, object] = {}
LAST_RESULTS = None  # BassKernelResults of the most recent run (for test harness)
TRACE = False        # test harness can set kernel.TRACE = True for profiling
ACT_OVERRIDE = None  # sim-only: CoreSim lacks Gelu; tests may set e.g. "Relu"
LAST_CALL = None     # (nc, in_maps) of the most recent run, for re-runs
WARM_N = 16          # HAM/pstate pre-warm zero-matmuls at kernel start


def _seg_blocks(A: int, first: int | None = None):
    """Split A token columns into near-equal blocks <= 512 (PSUM bank).

    first: size of the first block (kept >= ~240 so LDWEIGHTS stays
    hidden); used for segment 0 so its first PSUM group needs less DMA.
    """
    blocks = []
    t = 0
    if first is not None and A > first + 240:
        blocks.append((0, first))
        t = first
        A -= first
    nblk = -(-A // 512)
    base = A // nblk
    rem = A % nblk
    for i in range(nblk):
        nb = base + (1 if i < rem else 0)
        blocks.append((t, nb))
        t += nb
    return blocks


def _build_program(segs: tuple[int, ...]):
    """Build + compile the per-core SPMD Bass program.

    segs: token capacity per segment (same on all cores; group token
    counts are padded up to these shared slot sizes).

    DRAM I/O (S = len(segs), Ctot = sum(segs)):
      xw  [H, S*FFS + Ctot] bf16  per segment: [w1 shard | x tokens^T]
      w2  [S*FFS, H]  bf16  per-core W2 shards
      b1p [P, S*KFS]  f32   b1 shard, col ff = b1[ff*128:(ff+1)*128]
      y   [H, Ctot]   bf16  partial YT (unscaled, host sums per group)
    """
    from contextlib import ExitStack

    from concourse import bacc
    import concourse.mybir as mybir
    import concourse.tile as tile

    dt = mybir.dt
    S = len(segs)
    Ctot = sum(segs)
    CSMAX = max(segs)

    def blocks_for(si):
        return _seg_blocks(segs[si])

    NBMAX = max(nb for si in range(S) for _, nb in blocks_for(si))

    nc = bacc.Bacc(None, target_bir_lowering=False, debug=False)

    xw = nc.dram_tensor("xw", [H, S * FFS + Ctot], dt.bfloat16,
                        kind="ExternalInput")
    w2 = nc.dram_tensor("w2", [S * FFS, H], dt.bfloat16, kind="ExternalInput")
    b1p = nc.dram_tensor("b1p", [P, S * KFS], dt.float32, kind="ExternalInput")
    y = nc.dram_tensor("y", [H, Ctot], dt.bfloat16, kind="ExternalOutput")

    xw_r = xw[:, :].rearrange("(k p) t -> p k t", p=P)
    y_r = y[:, :].rearrange("(k p) t -> p k t", p=P)
    seg_off = [0]   # xw column offset of segment si's [w1 | x] span
    out_off = [0]   # y column offset of segment si
    for A in segs:
        seg_off.append(seg_off[-1] + FFS + A)
        out_off.append(out_off[-1] + A)

    with ExitStack() as ctx:
        tc = ctx.enter_context(tile.TileContext(nc))
        xwpool = ctx.enter_context(tc.tile_pool(name="xwpool", bufs=2))
        w2pool = ctx.enter_context(tc.tile_pool(name="w2pool", bufs=2))
        bpool = ctx.enter_context(tc.tile_pool(name="bpool", bufs=1))
        hpool = ctx.enter_context(tc.tile_pool(name="hpool", bufs=2))
        psA = ctx.enter_context(tc.tile_pool(name="psA", bufs=4, space="PSUM"))
        psB = ctx.enter_context(tc.tile_pool(name="psB", bufs=4, space="PSUM"))
        opool = ctx.enter_context(tc.tile_pool(name="opool", bufs=3))

        act = getattr(mybir.ActivationFunctionType, ACT_OVERRIDE or "Gelu")
        tiles = {}

        b1t = bpool.tile([P, S * KFS], dt.float32, tag="b1t", name="b1t")

        def emit_inputs(si):
            """Issue segment si's input DMAs in consumption order."""
            A = segs[si]
            c0 = seg_off[si]
            nb0 = blocks_for(si)[0][1]
            ct = xwpool.tile([P, KH, FFS + CSMAX], dt.bfloat16, tag="ct",
                             name=f"ct{si}")
            w2t = w2pool.tile([P, KFS, H], dt.bfloat16, tag="w2t",
                              name=f"w2t{si}")
            tiles[si] = (ct, w2t)
            w2_r = w2[si * FFS:(si + 1) * FFS, :].rearrange(
                "(k p) h -> p k h", p=P)

            def cslice(a, b):
                nc.sync.dma_start(out=ct[:, :, a:b], in_=xw_r[:, :, c0 + a:c0 + b])

            if si == 0:
                # fine-grained: w1 ff-chunk 0, x block 0 (unblocks the first
                # PSUM group), then the rest in GEMM1 consumption order;
                # few slices - each DMA issue costs ~1us on the sync engine.
                # b1 (16KB) rides right behind the critical head slices: it
                # gates the first activation, hence the ff0-group eviction.
                cslice(0, P)
                cslice(FFS, FFS + nb0)
                nc.sync.dma_start(out=b1t[:], in_=b1p[:, :])
                cslice(P, 2 * P)
                cslice(2 * P, 4 * P)
                cslice(4 * P, FFS)
                if A > nb0:
                    cslice(FFS + nb0, FFS + A)
            else:
                cslice(0, FFS + A)
            nc.sync.dma_start(out=w2t[:, :, :], in_=w2_r[:, :, :])

        emit_inputs(0)
        if S > 1:
            emit_inputs(1)

        for si, A in enumerate(segs):
            o0 = out_off[si]
            blocks = blocks_for(si)
            ct, w2t = tiles.pop(si)

            # --- GEMM1: HmidT[f, t] = gelu(sum_k W1[h,f]*xt[h,t] + b1[f]) -
            hblk = hpool.tile([P, KFS, CSMAX], dt.bfloat16, tag="hblk",
                              name=f"hblk{si}")
            for bi, (t0, nb) in enumerate(blocks):
                for ff in range(KFS):
                    pa = psA.tile([P, NBMAX], dt.float32, tag="pa",
                                  name=f"pa{si}_{bi}_{ff}")
                    warm_n = 0
                    if si == 0 and bi == 0 and ff == 0:
                        # Pre-warm: accumulate zero-matmuls into the first
                        # PSUM group while the first input DMAs land; also
                        # ramps the PE clock out of its cold p-state.
                        warm_n = WARM_N
                        warm = bpool.tile([P, NBMAX], dt.bfloat16, tag="warm",
                                          name="warm")
                        nc.vector.memset(warm[:, :nb], 0.0)
                        for i in range(warm_n):
                            nc.tensor.matmul(
                                pa[:, :nb],
                                lhsT=warm[:, :P],
                                rhs=warm[:, :nb],
                                start=(i == 0),
                                stop=False,
                            )
                    for k in range(KH):
                        nc.tensor.matmul(
                            pa[:, :nb],
                            lhsT=ct[:, k, ff * P:(ff + 1) * P],
                            rhs=ct[:, k, FFS + t0:FFS + t0 + nb],
                            start=(k == 0 and warm_n == 0),
                            stop=(k == KH - 1),
                        )
                    nc.scalar.activation(
                        hblk[:, ff, t0:t0 + nb],
                        pa[:, :nb],
                        act,
                        bias=b1t[:, si * KFS + ff:si * KFS + ff + 1],
                    )

            # Prefetch segment si+1 inputs while GEMM2(si) computes.
            if si + 1 < S and si > 0:
                emit_inputs(si + 1)

            # --- GEMM2: YT[h, t] = sum_f W2[f, h] * HmidT[f, t] -----------
            for bi, (t0, nb) in enumerate(blocks):
                ot = opool.tile([P, KH, NBMAX], dt.bfloat16, tag="ot",
                                name=f"ot{si}_{bi}")
                last_blk = (si == S - 1 and bi == len(blocks) - 1)
                for ht in range(KH):
                    pb = psB.tile([P, NBMAX], dt.float32, tag="pb",
                                  name=f"pb{si}_{bi}_{ht}")
                    for k in range(KFS):
                        nc.tensor.matmul(
                            pb[:, :nb],
                            lhsT=w2t[:, k, ht * P:(ht + 1) * P],
                            rhs=hblk[:, k, t0:t0 + nb],
                            start=(k == 0),
                            stop=(k == KFS - 1),
                        )
                    nc.vector.tensor_copy(ot[:, ht, :nb], pb[:, :nb])
                    if last_blk and (ht in (1, 3, 5) or ht >= 6):
                        # tail: stagger the final block's output DMA, with
                        # the last two chunks per-ht so only ~1 ht-chunk of
                        # transfer trails the last matmul
                        lo = ht - 1 if ht in (1, 3, 5) else ht
                        nc.sync.dma_start(
                            out=y_r[:, lo:ht + 1, o0 + t0:o0 + t0 + nb],
                            in_=ot[:, lo:ht + 1, :nb],
                        )
                if not last_blk:
                    nc.sync.dma_start(
                        out=y_r[:, :, o0 + t0:o0 + t0 + nb],
                        in_=ot[:, :, :nb],
                    )

    nc.compile()
    return nc


def _get_program(segs: tuple[int, ...]):
    if segs not in _prog_cache:
        _prog_cache[segs] = _build_program(segs)
    return _prog_cache[segs]


def _route(xf: np.ndarray, Wg: np.ndarray, bg: np.ndarray):
    """fp32 gate: softmax -> top-2 (stable order, matches jax top_k) -> renorm."""
    logits = xf @ np.asarray(Wg, np.float32) + np.asarray(bg, np.float32)
    m = logits.max(axis=1, keepdims=True)
    p = np.exp(logits - m, dtype=np.float32)
    p /= p.sum(axis=1, keepdims=True)
    order = np.argsort(-p, axis=1, kind="stable")
    idx = order[:, :TOPK]
    pv = np.take_along_axis(p, idx, axis=1)
    vals = (pv / pv.sum(axis=1, keepdims=True)).astype(np.float32)
    return idx, vals


def kernel(x, Wg, bg, W1, b1, W2, b2):
    global LAST_RESULTS, LAST_CALL
    from concourse.bass_utils import run_bass_kernel_spmd

    x = np.asarray(x, np.float32)
    xf = x.reshape(-1, H)
    T = xf.shape[0]

    idx, vals = _route(xf, Wg, bg)
    counts = np.bincount(idx.ravel(), minlength=E)

    # Pair experts by sorted rank (1st with 2nd, 3rd with 4th, ...); each
    # pair contributes one slot sized max(pair). The pair members go to
    # different groups, larger one to the group with the smaller running
    # sum. Segments are ordered by slot size (desc) so the last segment
    # (drain tail) is the smallest.
    order = [int(e) for e in np.argsort(-counts, kind="stable")]
    gexp = [[], []]   # group -> expert per slot
    gsum = [0, 0]
    segs = []
    for si in range(GE):
        a, b = order[2 * si], order[2 * si + 1]
        if counts[a] == 0:
            break     # this pair (and all later ones) has no tokens
        segs.append(int(counts[a]))
        lo = 0 if gsum[0] <= gsum[1] else 1
        gexp[lo].append(a)
        gexp[1 - lo].append(b)
        gsum[lo] += int(counts[a])
        gsum[1 - lo] += int(counts[b])
    segs = tuple(segs)

    nc = _get_program(segs)

    bf16 = ml_dtypes.bfloat16
    W1 = np.asarray(W1, np.float32)
    W2 = np.asarray(W2, np.float32)
    b1 = np.asarray(b1, np.float32)
    S = len(segs)
    Ctot = sum(segs)

    # per (group, slot): token ids, combine scales, x^T padded to slot size
    shards = [[], []]
    xparts = [[], []]
    for g in range(NG):
        for si in range(S):
            e = gexp[g][si]
            sel = idx == e                  # [T, 2]; at most one True per row
            ids = np.nonzero(sel.any(axis=1))[0]
            sc = vals[sel]                  # row-major => aligned with ids
            shards[g].append((ids, sc))
            xp = np.zeros((H, segs[si]), bf16)
            xp[:, :ids.size] = xf[ids].T.astype(bf16)
            xparts[g].append(xp)

    in_maps = []
    for c in range(E):
        g, j = divmod(c, GE)
        pieces = []
        for si in range(S):
            e = gexp[g][si]
            pieces.append(W1[e][:, j * FFS:(j + 1) * FFS].astype(bf16))
            pieces.append(xparts[g][si])
        xwc = np.ascontiguousarray(np.concatenate(pieces, axis=1))
        w2c = np.concatenate(
            [W2[gexp[g][si]][j * FFS:(j + 1) * FFS, :] for si in range(S)],
            axis=0,
        ).astype(bf16)
        b1c = np.ascontiguousarray(np.concatenate(
            [b1[gexp[g][si]][j * FFS:(j + 1) * FFS].reshape(KFS, P).T
             for si in range(S)],
            axis=1,
        ))
        in_maps.append({"xw": xwc, "w2": w2c, "b1p": b1c})

    LAST_CALL = (nc, in_maps)
    LAST_RESULTS = run_bass_kernel_spmd(nc, in_maps, list(range(E)), trace=TRACE)

    out = np.zeros((T, H), np.float32)
    for g in range(NG):
        ysum = np.zeros((H, Ctot), np.float32)
        for c in range(g * GE, (g + 1) * GE):
            ysum += LAST_RESULTS.results[c]["y"].astype(np.float32)
        c0 = 0
        for si in range(S):
            ids, sc = shards[g][si]
            out[ids] += ysum[:, c0:c0 + ids.size].T * sc[:, None]
            c0 += segs[si]

    b2 = np.asarray(b2, np.float32)
    out += vals[:, 0:1] * b2[idx[:, 0]] + vals[:, 1:2] * b2[idx[:, 1]]
    return out.reshape(x.shape)



# revision 4
# speedup vs baseline: 1.1889x; 1.1889x over previous
"""FF-sharded MoE FFN kernel for Trainium2 (8 NeuronCores), v2 "W8".

Strategy (pure FF-tensor-parallel, single group):
  - Host computes the gate in fp32 (softmax -> top-2 -> renormalize).
  - Every core processes ALL routed (expert, token) visits; the FFN
    hidden dim (FF=4096) is sharded 8 ways: core c holds columns
    [c*512, (c+1)*512) of every expert's W1 and the matching rows of
    W2, and computes
        Ypart = gelu(X @ W1[:, shard] + b1[shard]) @ W2[shard, :]
    for each expert segment. The host sums the 8 partials, applies the
    top-2 combine weights, and adds the b2 term.
  - Why: per-core work is exactly sum(counts)/8 * H * FFS MAC columns
    for ANY routing - zero load imbalance and zero slot padding (the
    previous expert-pairing scheme padded ~1%). HBM traffic is
    ~50MB/core (16 W + 17 x + 17 y), well under the ~95us of DMA a
    ~265us all-matmul kernel can hide.

Per-core schedule (8 segments = experts, descending token count):
  Segment inputs live in ONE DRAM tensor packed PARTITION-MAJOR: for
  each SBUF partition p, each segment's [w1 shard | x^T] block is a
  single contiguous run ordered [k][col] (k = contraction chunk). A
  full-segment input DMA is therefore 128 descriptors of ~25KB - near
  peak HBM bandwidth (the previous [H, cols] layout produced 1024
  256B-4KB descriptors; 256B descriptors measured only ~48GB/s).
  Segment 0 is split into 4 separately-DMA'd tiles (w1 ff-chunk 0,
  x block 0, w1 ff-chunks 1-3, x rest) so the first GEMM can start
  ~3us earlier; zero-matmul warm-up rides the initial DMA wait and
  ramps the PE out of its cold HAM state.
  All GEMMs in bf16 on the PE with fp32 PSUM accumulation; gelu
  (exact) is fused into the GEMM1 PSUM eviction with the b1 bias;
  GEMM2 partial outputs are written as bf16 (host sums in fp32). The
  last block's output DMA is staggered per-128-row chunk so only a
  ~0.2MB transfer trails the final matmul.
"""

import sys

if "/opt/trn_rl_repo" not in sys.path:
    sys.path.insert(0, "/opt/trn_rl_repo")

import numpy as np
import ml_dtypes

H = 1024          # hidden size
E = 8             # experts
TOPK = 2
FF = 4 * H        # expert hidden dim
P = 128           # SBUF partitions
NC = 8            # cores == FF shards
FFS = FF // NC    # per-core FF shard (512)
KH = H // P       # 8  contraction chunks for GEMM1
KFS = FFS // P    # 4  contraction chunks for GEMM2 (shard)
NB0 = 256         # segment-0 first block (small => fast head DMA)

_prog_cache: dict[tuple, object] = {}
LAST_RESULTS = None  # BassKernelResults of the most recent run (for test harness)
TRACE = False        # test harness can set kernel.TRACE = True for profiling
ACT_OVERRIDE = None  # sim-only: CoreSim lacks Gelu; tests may set e.g. "Relu"
LAST_CALL = None     # (nc, in_maps) of the most recent run, for re-runs
WARM_N = 8           # HAM/pstate pre-warm zero-matmuls at kernel start


def _seg_blocks(A: int, first: int | None = None):
    """Split A token columns into near-equal blocks <= 512.

    first: size of the first block (segment 0 only; small so its DMA
    lands early). Avoid blocks < ~230: below that LDWEIGHTS (~114ns)
    stops hiding behind the matmul stream.
    """
    blocks = []
    t = 0
    if first is not None:
        first = min(first, A)
        blocks.append((0, first))
        t = first
        A -= first
    if A > 0:
        nblk = -(-A // 512)
        base = A // nblk
        rem = A % nblk
        for i in range(nblk):
            nb = base + (1 if i < rem else 0)
            blocks.append((t, nb))
            t += nb
    return blocks


def _build_program(segs: tuple[int, ...]):
    """Build + compile the per-core SPMD Bass program.

    segs: token count per segment, descending (exact per-expert counts;
    identical on all cores).

    DRAM I/O (S = len(segs), Ctot = sum(segs)):
      xw  [P, 8*(S*FFS + Ctot)] bf16  partition-major packed inputs:
          per partition, per segment: [k][w1 cols | x cols] contiguous
          (segment 0 reordered into its 4 head chunks, see below)
      w2  [P, S*KFS*H] bf16  partition-major W2 shards: per partition,
          per segment: [k][h] contiguous
      b1p [P, S*KFS]  f32   b1 shard, col si*KFS+f = b1[f*128:(f+1)*128]
      y   [H, Ctot]   bf16  partial YT (unscaled, host sums all cores)
    """
    from contextlib import ExitStack

    from concourse import bacc
    import concourse.mybir as mybir
    import concourse.tile as tile

    dt = mybir.dt
    S = len(segs)
    Ctot = sum(segs)
    A0 = segs[0]
    nb0 = min(NB0, A0)

    def blocks_for(si):
        if si == 0:
            return _seg_blocks(segs[si], first=nb0)
        return _seg_blocks(segs[si])

    NBMAX = max(nb for si in range(S) for _, nb in blocks_for(si))

    nc = bacc.Bacc(None, target_bir_lowering=False, debug=False)

    xw = nc.dram_tensor("xw", [P, KH * (S * FFS + Ctot)], dt.bfloat16,
                        kind="ExternalInput")
    w2 = nc.dram_tensor("w2", [P, S * KFS * H], dt.bfloat16,
                        kind="ExternalInput")
    b1p = nc.dram_tensor("b1p", [P, S * KFS], dt.float32, kind="ExternalInput")
    y = nc.dram_tensor("y", [H, Ctot], dt.bfloat16, kind="ExternalOutput")

    y_r = y[:, :].rearrange("(k p) t -> p k t", p=P)

    # xw element offset (per partition) of each segment's packed block
    seg_off = [0]
    out_off = [0]
    for A in segs:
        seg_off.append(seg_off[-1] + KH * (FFS + A))
        out_off.append(out_off[-1] + A)

    def xw_src(seg_elem_off: int, ncols: int):
        """[p, k, c] view of a contiguous per-partition run of xw."""
        a = seg_elem_off
        return xw[:, a:a + KH * ncols].rearrange("p (k c) -> p k c", k=KH)

    with ExitStack() as ctx:
        tc = ctx.enter_context(tile.TileContext(nc))
        xwpool = ctx.enter_context(tc.tile_pool(name="xwpool", bufs=2))
        w2pool = ctx.enter_context(tc.tile_pool(name="w2pool", bufs=2))
        bpool = ctx.enter_context(tc.tile_pool(name="bpool", bufs=1))
        hpool = ctx.enter_context(tc.tile_pool(name="hpool", bufs=2))
        psA = ctx.enter_context(tc.tile_pool(name="psA", bufs=4, space="PSUM"))
        psB = ctx.enter_context(tc.tile_pool(name="psB", bufs=4, space="PSUM"))
        opool = ctx.enter_context(tc.tile_pool(name="opool", bufs=3))

        act = getattr(mybir.ActivationFunctionType, ACT_OVERRIDE or "Gelu")
        tiles = {}

        b1t = bpool.tile([P, S * KFS], dt.float32, tag="b1t", name="b1t")

        # --- segment 0: 4 one-shot tiles, DMA'd in consumption order ---
        # xw layout for seg 0 (per partition, element offsets from 0):
        #   ctW  [k][128]      w1 ff-chunk 0
        #   ctX  [k][nb0]      x block 0
        #   ctB1 [k][FFS-128]  w1 ff-chunks 1..KFS-1
        #   ctB2 [k][A0-nb0]   x rest
        ctW = bpool.tile([P, KH, P], dt.bfloat16, tag="ctW", name="ctW")
        ctX = bpool.tile([P, KH, nb0], dt.bfloat16, tag="ctX", name="ctX")
        ctB1 = bpool.tile([P, KH, FFS - P], dt.bfloat16, tag="ctB1",
                          name="ctB1")
        ctB2 = None
        if A0 > nb0:
            ctB2 = bpool.tile([P, KH, A0 - nb0], dt.bfloat16, tag="ctB2",
                              name="ctB2")

        def emit_seg0():
            o = 0
            nc.sync.dma_start(out=ctW[:, :, :], in_=xw_src(o, P))
            o += KH * P
            nc.sync.dma_start(out=ctX[:, :, :], in_=xw_src(o, nb0))
            o += KH * nb0
            nc.sync.dma_start(out=b1t[:], in_=b1p[:, :])
            nc.sync.dma_start(out=ctB1[:, :, :], in_=xw_src(o, FFS - P))
            o += KH * (FFS - P)
            if ctB2 is not None:
                nc.sync.dma_start(out=ctB2[:, :, :], in_=xw_src(o, A0 - nb0))
            emit_w2(0)

        def emit_w2(si):
            w2t = w2pool.tile([P, KFS, H], dt.bfloat16, tag="w2t",
                              name=f"w2t{si}")
            tiles[("w2", si)] = w2t
            src = w2[:, si * KFS * H:(si + 1) * KFS * H].rearrange(
                "p (k h) -> p k h", k=KFS)
            nc.sync.dma_start(out=w2t[:, :, :], in_=src)

        def emit_seg(si):
            A = segs[si]
            ct = xwpool.tile([P, KH, FFS + A], dt.bfloat16, tag="ct",
                             name=f"ct{si}")
            tiles[("ct", si)] = ct
            nc.sync.dma_start(out=ct[:, :, :], in_=xw_src(seg_off[si], FFS + A))
            emit_w2(si)

        def lhsT1(si, k, ff):
            """GEMM1 stationary operand: w1 ff-chunk (128 cols)."""
            if si == 0:
                if ff == 0:
                    return ctW[:, k, :]
                return ctB1[:, k, (ff - 1) * P:ff * P]
            ct = tiles[("ct", si)]
            return ct[:, k, ff * P:(ff + 1) * P]

        def rhs1(si, k, t0, nb):
            """GEMM1 moving operand: x token block."""
            if si == 0:
                if t0 < nb0:
                    return ctX[:, k, t0:t0 + nb]
                return ctB2[:, k, t0 - nb0:t0 - nb0 + nb]
            ct = tiles[("ct", si)]
            return ct[:, k, FFS + t0:FFS + t0 + nb]

        emit_seg0()
        if S > 1:
            emit_seg(1)

        for si, A in enumerate(segs):
            o0 = out_off[si]
            blocks = blocks_for(si)

            # --- GEMM1: HmidT[f, t] = gelu(sum_k W1[h,f]*xt[h,t] + b1[f])
            hblk = hpool.tile([P, KFS, A], dt.bfloat16, tag="hblk",
                              name=f"hblk{si}")
            for bi, (t0, nb) in enumerate(blocks):
                for ff in range(KFS):
                    pa = psA.tile([P, NBMAX], dt.float32, tag="pa",
                                  name=f"pa{si}_{bi}_{ff}")
                    warm_n = 0
                    if si == 0 and bi == 0 and ff == 0:
                        # Pre-warm: accumulate zero-matmuls into the first
                        # PSUM group while the first input DMAs land; also
                        # ramps the PE clock out of its cold p-state.
                        warm_n = WARM_N
                        warm = bpool.tile([P, NBMAX], dt.bfloat16, tag="warm",
                                          name="warm")
                        nc.vector.memset(warm[:, :nb], 0.0)
                        for i in range(warm_n):
                            nc.tensor.matmul(
                                pa[:, :nb],
                                lhsT=warm[:, :P],
                                rhs=warm[:, :nb],
                                start=(i == 0),
                                stop=False,
                            )
                    for k in range(KH):
                        nc.tensor.matmul(
                            pa[:, :nb],
                            lhsT=lhsT1(si, k, ff),
                            rhs=rhs1(si, k, t0, nb),
                            start=(k == 0 and warm_n == 0),
                            stop=(k == KH - 1),
                        )
                    nc.scalar.activation(
                        hblk[:, ff, t0:t0 + nb],
                        pa[:, :nb],
                        act,
                        bias=b1t[:, si * KFS + ff:si * KFS + ff + 1],
                    )
                if bi == 0 and si + 1 < S and si > 0:
                    # Prefetch segment si+1 while the rest of this segment
                    # computes (~28us of cover for ~4MB).
                    emit_seg(si + 1)

            # --- GEMM2: YT[h, t] = sum_f W2[f, h] * HmidT[f, t] -----------
            w2t = tiles.pop(("w2", si))
            for bi, (t0, nb) in enumerate(blocks):
                ot = opool.tile([P, KH, NBMAX], dt.bfloat16, tag="ot",
                                name=f"ot{si}_{bi}")
                last_blk = (si == S - 1 and bi == len(blocks) - 1)
                for ht in range(KH):
                    pb = psB.tile([P, NBMAX], dt.float32, tag="pb",
                                  name=f"pb{si}_{bi}_{ht}")
                    for k in range(KFS):
                        nc.tensor.matmul(
                            pb[:, :nb],
                            lhsT=w2t[:, k, ht * P:(ht + 1) * P],
                            rhs=hblk[:, k, t0:t0 + nb],
                            start=(k == 0),
                            stop=(k == KFS - 1),
                        )
                    nc.vector.tensor_copy(ot[:, ht, :nb], pb[:, :nb])
                    if last_blk and (ht == 3 or ht >= 4):
                        # tail: drain the final block per-ht so only one
                        # ~128-row chunk of transfer trails the last matmul
                        lo = 0 if ht == 3 else ht
                        nc.sync.dma_start(
                            out=y_r[:, lo:ht + 1, o0 + t0:o0 + t0 + nb],
                            in_=ot[:, lo:ht + 1, :nb],
                        )
                if not last_blk:
                    nc.sync.dma_start(
                        out=y_r[:, :, o0 + t0:o0 + t0 + nb],
                        in_=ot[:, :, :nb],
                    )

    nc.compile()
    return nc


def _get_program(segs: tuple[int, ...]):
    if segs not in _prog_cache:
        _prog_cache[segs] = _build_program(segs)
    return _prog_cache[segs]


def _route(xf: np.ndarray, Wg: np.ndarray, bg: np.ndarray):
    """fp32 gate: softmax -> top-2 (stable order, matches jax top_k) -> renorm."""
    logits = xf @ np.asarray(Wg, np.float32) + np.asarray(bg, np.float32)
    m = logits.max(axis=1, keepdims=True)
    p = np.exp(logits - m, dtype=np.float32)
    p /= p.sum(axis=1, keepdims=True)
    order = np.argsort(-p, axis=1, kind="stable")
    idx = order[:, :TOPK]
    pv = np.take_along_axis(p, idx, axis=1)
    vals = (pv / pv.sum(axis=1, keepdims=True)).astype(np.float32)
    return idx, vals


def _pack_pm(arr_hc: np.ndarray) -> np.ndarray:
    """[H, C] -> partition-major [P, KH*C] (per partition: [k][c])."""
    h, c = arr_hc.shape
    return np.ascontiguousarray(
        arr_hc.reshape(h // P, P, c).transpose(1, 0, 2).reshape(P, -1)
    )


def kernel(x, Wg, bg, W1, b1, W2, b2):
    global LAST_RESULTS, LAST_CALL
    from concourse.bass_utils import run_bass_kernel_spmd

    bf16 = ml_dtypes.bfloat16
    x = np.asarray(x, np.float32)
    xf = x.reshape(-1, H)
    T = xf.shape[0]

    idx, vals = _route(xf, Wg, bg)
    counts = np.bincount(idx.ravel(), minlength=E)

    # Segments: experts by token count (desc), zero-count experts skipped.
    order = [int(e) for e in np.argsort(-counts, kind="stable") if counts[e] > 0]
    segs = tuple(int(counts[e]) for e in order)
    S = len(segs)
    Ctot = sum(segs)
    A0 = segs[0]
    nb0 = min(NB0, A0)

    nc = _get_program(segs)

    W1 = np.asarray(W1, np.float32)
    W2 = np.asarray(W2, np.float32)
    b1 = np.asarray(b1, np.float32)

    # Token ids / combine scales / packed x^T per segment (shared by cores).
    shards = []
    xparts = []   # per segment: [P, KH*A] partition-major bf16
    for si in range(S):
        e = order[si]
        sel = idx == e                  # [T, 2]; at most one True per row
        ids = np.nonzero(sel.any(axis=1))[0]
        sc = vals[sel]                  # row-major => aligned with ids
        shards.append((ids, sc))
        xparts.append(_pack_pm(xf[ids].T.astype(bf16)))

    in_maps = []
    for c in range(NC):
        pieces = []
        for si in range(S):
            e = order[si]
            w1s = W1[e][:, c * FFS:(c + 1) * FFS].astype(bf16)
            if si == 0:
                # head chunk order: w1 ff0 | x blk0 | w1 ff1.. | x rest
                x3 = xparts[0].reshape(P, KH, A0)
                pieces.append(_pack_pm(w1s[:, :P]))
                pieces.append(np.ascontiguousarray(x3[:, :, :nb0])
                              .reshape(P, -1))
                pieces.append(_pack_pm(w1s[:, P:]))
                if A0 > nb0:
                    pieces.append(np.ascontiguousarray(x3[:, :, nb0:])
                                  .reshape(P, -1))
            else:
                # per partition: [k][w1 cols | x cols] contiguous
                w13 = _pack_pm(w1s).reshape(P, KH, FFS)
                x3 = xparts[si].reshape(P, KH, segs[si])
                pieces.append(np.concatenate([w13, x3], axis=2)
                              .reshape(P, -1))
        xwc = np.ascontiguousarray(np.concatenate(pieces, axis=1))
        w2c = np.concatenate(
            [_pack_pm(W2[order[si]][c * FFS:(c + 1) * FFS, :].astype(bf16))
             for si in range(S)],
            axis=1,
        )
        b1c = np.ascontiguousarray(np.stack(
            [b1[order[si]][c * FFS + f * P:c * FFS + (f + 1) * P]
             for si in range(S) for f in range(KFS)],
            axis=1,
        ))
        in_maps.append({"xw": xwc, "w2": np.ascontiguousarray(w2c), "b1p": b1c})

    LAST_CALL = (nc, in_maps)
    LAST_RESULTS = run_bass_kernel_spmd(nc, in_maps, list(range(NC)),
                                        trace=TRACE)

    ysum = np.zeros((H, Ctot), np.float32)
    for c in range(NC):
        ysum += LAST_RESULTS.results[c]["y"].astype(np.float32)

    out = np.zeros((T, H), np.float32)
    c0 = 0
    for si in range(S):
        ids, sc = shards[si]
        out[ids] += ysum[:, c0:c0 + ids.size].T * sc[:, None]
        c0 += segs[si]

    b2 = np.asarray(b2, np.float32)
    out += vals[:, 0:1] * b2[idx[:, 0]] + vals[:, 1:2] * b2[idx[:, 1]]
    return out.reshape(x.shape)
